# revision 8
# baseline (speedup 1.0000x reference)
"""Multi-head self-attention (B=2, N=4096, D=512, h=8, d=64) on 8 TRN2 cores.

Sharding: batch*head-pair across the 8 cores (core c -> batch c//4, heads
2*(c%4), 2*(c%4)+1). Each core computes its two heads' q/k/v projections,
flash-style attention (scores kept transposed [j, i] so no P-matrix
transposes are ever needed), and its partial output projection
pT = (out_heads/denom)^T @ Wo_rows. Host sums the 4 partials per batch
and adds bo. No cross-core communication.
"""

import numpy as np

import concourse.bass as bass
import concourse.tile as tile
from concourse import bacc, mybir
from concourse.bass_utils import run_bass_kernel_spmd
from concourse.masks import make_identity

F32 = mybir.dt.float32
F32R = mybir.dt.float32r
BF16 = mybir.dt.bfloat16

B, N, D = 2, 4096, 512
HEADS, DH = 8, 64
SCALE = DH ** -0.5          # 0.125
IC = 1024                   # i-chunk (query cols per psum-out accumulation)
JC = 128                    # j-chunk (key rows per matmul = partitions)
N_IC = N // IC              # 4
N_JC = N // JC              # 32
N_CORES = 8


def _r(ap):
    return ap.bitcast(F32R)


def build_kernel():
    nc = bacc.Bacc("TRN2", target_bir_lowering=False, debug=False)
    xT_d = nc.dram_tensor("xT", [D, N], F32R, kind="ExternalInput").ap()
    wq_d = nc.dram_tensor("wq", [D, 128], F32R, kind="ExternalInput").ap()
    wk_d = nc.dram_tensor("wk", [D, 128], F32R, kind="ExternalInput").ap()
    wv_d = nc.dram_tensor("wv", [D, 128], F32R, kind="ExternalInput").ap()
    wo_d = nc.dram_tensor("wo", [128, D], F32R, kind="ExternalInput").ap()
    pT_d = nc.dram_tensor("pT", [D, N], F32, kind="ExternalOutput").ap()

    with tile.TileContext(nc) as tc:
        with (
            tc.tile_pool(name="const", bufs=1) as const_pool,
            tc.tile_pool(name="proj", bufs=1) as proj_pool,
            tc.tile_pool(name="pt", bufs=3) as pt_pool,
            tc.tile_pool(name="norm", bufs=2) as norm_pool,
            tc.tile_pool(name="stage", bufs=3) as stage_pool,
            tc.tile_pool(name="ps", bufs=2, space="PSUM") as ps_pool,
            tc.tile_pool(name="po", bufs=2, space="PSUM") as po_pool,
        ):
            # ---- P0: loads + constants -------------------------------------
            xt_sb = []
            for dc in range(4):
                t = const_pool.tile([128, N], F32R, name=f"xt{dc}", tag=f"xt{dc}")
                nc.sync.dma_start(t[:], xT_d[dc * 128:(dc + 1) * 128, :])
                xt_sb.append(t)
            w_sb = {}
            for nm, d_ap in (("wq", wq_d), ("wk", wk_d), ("wv", wv_d)):
                t = const_pool.tile([128, 4, 128], F32R, name=f"{nm}s", tag=f"{nm}s")
                nc.sync.dma_start(t[:], d_ap.rearrange("(c p) e -> p c e", p=128))
                w_sb[nm] = t
            wo_sb = const_pool.tile([128, D], F32R, name="wos", tag="wos")
            nc.sync.dma_start(wo_sb[:], wo_d[:])
            ident_f = const_pool.tile([128, 128], F32, name="ident_f",
                                      tag="ident_f")
            make_identity(nc, ident_f[:])
            ident = const_pool.tile([128, 128], F32R, name="ident", tag="ident")
            nc.vector.tensor_copy(ident[:], ident_f[:])
            # only row 64 is used (as a [1, 64] ones stationary aligned with
            # the denominator row's base partition)
            # e1: [64, 128] with 1 at [k, 64+k] — shifts head-1 rows to
            # partitions 64..127 in the stack matmul
            e1_f = const_pool.tile([64, 128], F32, name="e1_f", tag="e1_f")
            nc.gpsimd.memset(e1_f[:], 0.0)
            nc.gpsimd.affine_select(
                out=e1_f[:], in_=e1_f[:],
                compare_op=mybir.AluOpType.not_equal, fill=1.0,
                base=64, pattern=[[-1, 128]], channel_multiplier=1,
            )
            e1 = const_pool.tile([64, 128], F32R, name="e1", tag="e1")
            nc.vector.tensor_copy(e1[:], e1_f[:])
            # msel: [65, 256]; row 64 selects the denominator row into
            # partitions 0..63 (block 0) / 64..127 (block 1)
            msel_f = const_pool.tile([65, 256], F32, name="msel_f", tag="msel_f")
            nc.vector.memset(msel_f[:], 0.0)
            nc.vector.memset(msel_f[64:65, 0:64], 1.0)
            nc.vector.memset(msel_f[64:65, 192:256], 1.0)
            msel = const_pool.tile([65, 256], F32R, name="msel", tag="msel")
            nc.vector.tensor_copy(msel[:], msel_f[:])

            # ---- P1: projections -------------------------------------------
            # qT2/kT2: [128 (2 heads x 64 dh), N] = W.T @ x.T
            qT2 = proj_pool.tile([128, N], F32R, name="qT2", tag="qT2")
            kT2 = proj_pool.tile([128, N], F32R, name="kT2", tag="kT2")
            vT2 = proj_pool.tile([128, N], F32R, name="vT2", tag="vT2")
            for dst, wname in ((kT2, "wk"), (vT2, "wv"), (qT2, "wq")):
                for i8 in range(8):
                    sl = slice(i8 * 512, (i8 + 1) * 512)
                    ps = ps_pool.tile([128, 1024], F32, name="ps", tag="ps")
                    for dc in range(4):
                        nc.tensor.matmul(
                            ps[:, 0:512],
                            w_sb[wname][:, dc, :],
                            xt_sb[dc][:, sl],
                            start=(dc == 0),
                            stop=(dc == 3),
                        )
                    nc.vector.tensor_copy(dst[:, sl], ps[:, 0:512])

            # v natural [j, e] in bf16, ones-augmented per head:
            # v2aug[:, jc, 0:64]=v_h0, [64]=1, [65:129]=v_h1, [129]=1
            v2aug = proj_pool.tile([128, N_JC, 130], BF16, name="v2aug", tag="v2aug")
            for jc in range(N_JC):
                ps = ps_pool.tile([128, 1024], F32R, name="ps", tag="ps")
                nc.tensor.transpose(
                    ps[:, 0:128], vT2[:, jc * 128:(jc + 1) * 128], ident[:]
                )
                nc.vector.tensor_copy(v2aug[:, jc, 0:64], ps[:, 0:64])
                nc.vector.tensor_copy(v2aug[:, jc, 65:129], ps[:, 64:128])
            nc.vector.memset(v2aug[:, :, 64:65], 1.0)
            nc.vector.memset(v2aug[:, :, 129:130], 1.0)

            # ---- P2+P3: attention + normalize + output projection ----------
            for ic in range(N_IC):
                isl = slice(ic * IC, (ic + 1) * IC)
                outu = []
                for h in range(2):
                    hsl = slice(h * 64, h * 64 + 64)
                    pout = po_pool.tile([65, IC], F32, name="pout", tag="po")
                    for jc in range(N_JC):
                        jsl = slice(jc * 128, (jc + 1) * 128)
                        sc = ps_pool.tile([128, IC], F32, name="sc", tag="ps")
                        for n2 in range(2):
                            nsl = slice(n2 * 512, (n2 + 1) * 512)
                            nc.tensor.matmul(
                                sc[:, nsl],
                                kT2[hsl, jsl],
                                qT2[hsl, ic * IC + n2 * 512:
                                    ic * IC + (n2 + 1) * 512],
                                start=True,
                                stop=True,
                            )
                        pt = pt_pool.tile([128, IC], BF16, name="pt", tag="pt")
                        nc.scalar.activation(
                            pt[:], sc[:], mybir.ActivationFunctionType.Exp,
                            scale=SCALE,
                        )
                        for n2 in range(2):
                            nsl = slice(n2 * 512, (n2 + 1) * 512)
                            nc.tensor.matmul(
                                pout[:, nsl],
                                v2aug[:, jc, h * 65:h * 65 + 65],
                                pt[:, nsl],
                                start=(jc == 0),
                                stop=(jc == N_JC - 1),
                            )
                    # rows 0..63 = unnormalized out^T, row 64 = denom
                    ou = norm_pool.tile([65, IC], F32R, name=f"outu{h}",
                                        tag=f"outu{h}")
                    nc.vector.tensor_copy(ou[:], pout[:])
                    outu.append(ou)

                # stack both heads on partitions 0..127 (PE shift), bcast denom
                stk = ps_pool.tile([128, IC], F32, name="stk", tag="ps")
                bcd = ps_pool.tile([128, IC], F32, name="bcd", tag="ps")
                for h in range(2):
                    lhs_stk = ident[0:64, 0:128] if h == 0 else e1[:, :]
                    for n2 in range(2):
                        nsl = slice(n2 * 512, (n2 + 1) * 512)
                        nc.tensor.matmul(
                            stk[:, nsl], lhs_stk, outu[h][0:64, nsl],
                            start=(h == 0), stop=(h == 1),
                        )
                        nc.tensor.matmul(
                            bcd[:, nsl], msel[0:65, h * 128:(h + 1) * 128],
                            outu[h][0:65, nsl],
                            start=(h == 0), stop=(h == 1),
                        )
                rec = norm_pool.tile([128, IC], F32, name="rec", tag="rec")
                nc.vector.reciprocal(rec[:], bcd[:])
                outn = norm_pool.tile([128, IC], F32R, name="outn", tag="outn")
                nc.vector.tensor_mul(outn[:], stk[:], rec[:])

                # partial out projection: pT[oc, i] = wo[:, oc].T @ outn[:, i]
                for oc in range(4):
                    for n2 in range(2):
                        nsl = slice(n2 * 512, (n2 + 1) * 512)
                        pp = ps_pool.tile([128, 1024], F32, name="pp", tag="ps")
                        nc.tensor.matmul(
                            pp[:, 0:512],
                            wo_sb[:, oc * 128:(oc + 1) * 128],
                            outn[:, nsl],
                            start=True, stop=True,
                        )
                        st = stage_pool.tile([128, 512], F32, name="st", tag="st")
                        nc.vector.tensor_copy(st[:], pp[:, 0:512])
                        nc.sync.dma_start(
                            pT_d[oc * 128:(oc + 1) * 128,
                                 ic * IC + n2 * 512:ic * IC + (n2 + 1) * 512],
                            st[:],
                        )
    nc.compile()
    return nc


_CACHE = {}


def _get_nc():
    if "nc" not in _CACHE:
        _CACHE["nc"] = build_kernel()
    return _CACHE["nc"]


def kernel(x, Wq, Wkv, Wo, bo):
    x = np.asarray(x, dtype=np.float32)
    Wq = np.asarray(Wq, dtype=np.float32)
    Wkv = np.asarray(Wkv, dtype=np.float32)
    Wo = np.asarray(Wo, dtype=np.float32)
    bo = np.asarray(bo, dtype=np.float32)

    nc = _get_nc()
    xTs = [np.ascontiguousarray(x[b].T) for b in range(B)]
    in_maps = []
    for c in range(N_CORES):
        b, p = divmod(c, 4)
        cs = slice(128 * p, 128 * (p + 1))
        in_maps.append({
            "xT": xTs[b],
            "wq": np.ascontiguousarray(Wq[:, cs]),
            "wk": np.ascontiguousarray(Wkv[:, :D][:, cs]),
            "wv": np.ascontiguousarray(Wkv[:, D:][:, cs]),
            "wo": np.ascontiguousarray(Wo[cs, :]),
        })
    res = run_bass_kernel_spmd(nc, in_maps, core_ids=list(range(N_CORES)))
    out = np.empty((B, N, D), dtype=np.float32)
    for b in range(B):
        acc = res.results[4 * b]["pT"].copy()
        for p in range(1, 4):
            acc += res.results[4 * b + p]["pT"]
        out[b] = acc.T + bo
    return out


# revision 14
# speedup vs baseline: 1.1100x; 1.1100x over previous
"""Multi-head self-attention (B=2, N=4096, D=512, h=8, d=64) on 8 TRN2 cores.

Sharding: batch*head-pair across the 8 cores (core c -> batch c//4, heads
2*(c%4), 2*(c%4)+1). Each core computes its two heads' q/k/v projections,
flash-style attention (scores kept transposed [j, i] so no P-matrix
transposes are ever needed; softmax denominators come from a ones-augmented
V stationary), and its partial output projection. Host sums the 4 partials
per batch and adds bo. No cross-core communication.

All matmuls run in bf16 (PE stays on the warm 2.4 GHz clock; fp32r's
transpose-mode path never warms the HAM clock gate and ran 2.7x slower).
"""

import numpy as np
import ml_dtypes

import concourse.bass as bass
import concourse.tile as tile
from concourse import bacc, mybir
from concourse.bass_utils import run_bass_kernel_spmd
from concourse.masks import make_identity

F32 = mybir.dt.float32
BF16 = mybir.dt.bfloat16

B, N, D = 2, 4096, 512
HEADS, DH = 8, 64
SCALE = DH ** -0.5          # 0.125
IC = 1024                   # i-chunk (query cols per psum-out accumulation)
N_IC = N // IC              # 4
N_JC = N // 128             # 32 j-chunks (key rows per matmul = partitions)
N_CORES = 8


def build_kernel():
    nc = bacc.Bacc("TRN2", target_bir_lowering=False, debug=False)
    xT_d = nc.dram_tensor("xT", [D, N], BF16, kind="ExternalInput").ap()
    wq_d = nc.dram_tensor("wq", [D, 128], BF16, kind="ExternalInput").ap()
    wk_d = nc.dram_tensor("wk", [D, 128], BF16, kind="ExternalInput").ap()
    wv_d = nc.dram_tensor("wv", [D, 128], BF16, kind="ExternalInput").ap()
    wo_d = nc.dram_tensor("wo", [128, D], BF16, kind="ExternalInput").ap()
    pT_d = nc.dram_tensor("pT", [D, N], F32, kind="ExternalOutput").ap()

    with tile.TileContext(nc) as tc:
        with (
            tc.tile_pool(name="const", bufs=1) as const_pool,
            tc.tile_pool(name="proj", bufs=1) as proj_pool,
            tc.tile_pool(name="pt", bufs=3) as pt_pool,
            tc.tile_pool(name="norm", bufs=2) as norm_pool,
            tc.tile_pool(name="stage", bufs=3) as stage_pool,
            tc.tile_pool(name="ps", bufs=2, space="PSUM") as ps_pool,
            tc.tile_pool(name="po", bufs=2, space="PSUM") as po_pool,
        ):
            # ---- P0: loads + constants -------------------------------------
            xt_sb = []
            for dc in range(4):
                t = const_pool.tile([128, N], BF16, name=f"xt{dc}", tag=f"xt{dc}")
                nc.sync.dma_start(t[:], xT_d[dc * 128:(dc + 1) * 128, :])
                xt_sb.append(t)
            w_sb = {}
            for nm, d_ap in (("wq", wq_d), ("wk", wk_d), ("wv", wv_d)):
                t = const_pool.tile([128, 4, 128], BF16, name=f"{nm}s", tag=f"{nm}s")
                nc.sync.dma_start(t[:], d_ap.rearrange("(c p) e -> p c e", p=128))
                w_sb[nm] = t
            wo_sb = const_pool.tile([128, D], BF16, name="wos", tag="wos")
            nc.sync.dma_start(wo_sb[:], wo_d[:])
            ident_f = const_pool.tile([128, 128], F32, name="ident_f",
                                      tag="ident_f")
            make_identity(nc, ident_f[:])
            ident = const_pool.tile([128, 128], BF16, name="ident", tag="ident")
            nc.vector.tensor_copy(ident[:], ident_f[:])

            # ---- P1: projections -------------------------------------------
            # qT2/kT2: [128 (2 heads x 64 dh), N] = W.T @ x.T   (bf16)
            qT2 = proj_pool.tile([128, N], BF16, name="qT2", tag="qT2")
            kT2 = proj_pool.tile([128, N], BF16, name="kT2", tag="kT2")
            vT2 = proj_pool.tile([128, N], BF16, name="vT2", tag="vT2")
            for dst, wname in ((kT2, "wk"), (vT2, "wv"), (qT2, "wq")):
                for i8 in range(8):
                    sl = slice(i8 * 512, (i8 + 1) * 512)
                    ps = ps_pool.tile([128, 1024], F32, name="ps", tag="ps")
                    for dc in range(4):
                        nc.tensor.matmul(
                            ps[:, 0:512],
                            w_sb[wname][:, dc, :],
                            xt_sb[dc][:, sl],
                            start=(dc == 0),
                            stop=(dc == 3),
                        )
                    nc.vector.tensor_copy(dst[:, sl], ps[:, 0:512])

            # v natural [j, e] in bf16, ones-augmented per head (ones column
            # FIRST so the softmax denominator lands on psum partition 0):
            # v2aug[:, jc, 0]=1, [1:65]=v_h0, [65]=1, [66:130]=v_h1
            v2aug = proj_pool.tile([128, N_JC, 130], BF16, name="v2aug",
                                   tag="v2aug")
            for jc in range(N_JC):
                psb = ps_pool.tile([128, 128], BF16, name="psb", tag="ps")
                nc.tensor.transpose(
                    psb[:, 0:128], vT2[:, jc * 128:(jc + 1) * 128], ident[:]
                )
                nc.vector.tensor_copy(v2aug[:, jc, 1:65], psb[:, 0:64])
                nc.vector.tensor_copy(v2aug[:, jc, 66:130], psb[:, 64:128])
            nc.vector.memset(v2aug[:, :, 0:1], 1.0)
            nc.vector.memset(v2aug[:, :, 65:66], 1.0)

            # ---- P2+P3: attention + normalize + output projection ----------
            for ic in range(N_IC):
                isl = slice(ic * IC, (ic + 1) * IC)
                outu = []
                for h in range(2):
                    hsl = slice(h * 64, h * 64 + 64)
                    pout = po_pool.tile([65, IC], F32, name="pout", tag="po")
                    for jc in range(N_JC):
                        jsl = slice(jc * 128, (jc + 1) * 128)
                        sc = ps_pool.tile([128, IC], F32, name="sc", tag="ps")
                        for n2 in range(2):
                            nsl = slice(n2 * 512, (n2 + 1) * 512)
                            nc.tensor.matmul(
                                sc[:, nsl],
                                kT2[hsl, jsl],
                                qT2[hsl, ic * IC + n2 * 512:
                                    ic * IC + (n2 + 1) * 512],
                                start=True,
                                stop=True,
                            )
                        pt = pt_pool.tile([128, IC], BF16, name="pt", tag="pt")
                        nc.scalar.activation(
                            pt[:], sc[:], mybir.ActivationFunctionType.Exp,
                            scale=SCALE,
                        )
                        for n2 in range(2):
                            nsl = slice(n2 * 512, (n2 + 1) * 512)
                            nc.tensor.matmul(
                                pout[:, nsl],
                                v2aug[:, jc, h * 65:h * 65 + 65],
                                pt[:, nsl],
                                start=(jc == 0),
                                stop=(jc == N_JC - 1),
                            )
                    # row 0 = denom, rows 1..64 = unnormalized out^T
                    ou = norm_pool.tile([65, IC], F32, name=f"outu{h}",
                                        tag=f"outu{h}")
                    nc.vector.tensor_copy(ou[:], pout[:])
                    outu.append(ou)

                # Normalize + stack both heads onto partitions 0..127:
                # denominators (row 0) partition-broadcast on GpSimd, out rows
                # (1..64) partition-shifted via SBUF->SBUF DMA, then
                # reciprocal + multiply on DVE.
                # partition_broadcast only writes correctly at out base 0, so
                # each head broadcasts into its own full-height tile and the
                # reciprocal reads the half that lines up with its out rows.
                den0 = norm_pool.tile([128, IC], F32, name="den0", tag="den0")
                den1 = norm_pool.tile([128, IC], F32, name="den1", tag="den1")
                st1 = norm_pool.tile([128, IC], F32, name="st1", tag="st1")
                for h, dtile in ((0, den0), (1, den1)):
                    psl = slice(h * 64, (h + 1) * 64)
                    nc.gpsimd.partition_broadcast(dtile[:, :], outu[h][0:1, :])
                    nc.sync.dma_start(st1[psl, :], outu[h][1:65, :])
                rec = norm_pool.tile([128, IC], F32, name="rec", tag="rec")
                nc.vector.reciprocal(rec[0:64, :], den0[0:64, :])
                nc.vector.reciprocal(rec[64:128, :], den1[64:128, :])
                outn = norm_pool.tile([128, IC], BF16, name="outn", tag="outn")
                nc.vector.tensor_mul(outn[:, :], st1[:, :], rec[:, :])

                # partial out projection: pT[oc, i] = wo[:, oc].T @ outn[:, i]
                for oc in range(4):
                    for n2 in range(2):
                        nsl = slice(n2 * 512, (n2 + 1) * 512)
                        pp = ps_pool.tile([128, 1024], F32, name="pp", tag="ps")
                        nc.tensor.matmul(
                            pp[:, 0:512],
                            wo_sb[:, oc * 128:(oc + 1) * 128],
                            outn[:, nsl],
                            start=True, stop=True,
                        )
                        st = stage_pool.tile([128, 512], F32, name="st",
                                             tag="st")
                        nc.vector.tensor_copy(st[:], pp[:, 0:512])
                        nc.sync.dma_start(
                            pT_d[oc * 128:(oc + 1) * 128,
                                 ic * IC + n2 * 512:ic * IC + (n2 + 1) * 512],
                            st[:],
                        )
    nc.compile()
    return nc


_CACHE = {}


def _get_nc():
    if "nc" not in _CACHE:
        _CACHE["nc"] = build_kernel()
    return _CACHE["nc"]


def make_in_map(x, Wq, Wkv, Wo, core):
    bf = ml_dtypes.bfloat16
    b, p = divmod(core, 4)
    cs = slice(128 * p, 128 * (p + 1))
    return {
        "xT": np.ascontiguousarray(x[b].T).astype(bf),
        "wq": np.ascontiguousarray(Wq[:, cs]).astype(bf),
        "wk": np.ascontiguousarray(Wkv[:, :D][:, cs]).astype(bf),
        "wv": np.ascontiguousarray(Wkv[:, D:][:, cs]).astype(bf),
        "wo": np.ascontiguousarray(Wo[cs, :]).astype(bf),
    }


def kernel(x, Wq, Wkv, Wo, bo):
    x = np.asarray(x, dtype=np.float32)
    Wq = np.asarray(Wq, dtype=np.float32)
    Wkv = np.asarray(Wkv, dtype=np.float32)
    Wo = np.asarray(Wo, dtype=np.float32)
    bo = np.asarray(bo, dtype=np.float32)

    nc = _get_nc()
    in_maps = [make_in_map(x, Wq, Wkv, Wo, c) for c in range(N_CORES)]
    res = run_bass_kernel_spmd(nc, in_maps, core_ids=list(range(N_CORES)))
    out = np.empty((B, N, D), dtype=np.float32)
    for b in range(B):
        acc = res.results[4 * b]["pT"].copy()
        for p in range(1, 4):
            acc += res.results[4 * b + p]["pT"]
        out[b] = acc.T + bo
    return out


# revision 16
# speedup vs baseline: 1.5776x; 1.4213x over previous
"""Multi-head self-attention (B=2, N=4096, D=512, h=8, d=64) on 8 TRN2 cores.

Sharding: batch*head-pair across the 8 cores (core c -> batch c//4, heads
2*(c%4), 2*(c%4)+1). Each core computes its two heads' q/k/v projections,
flash-style attention (scores kept transposed [j, i] so no P-matrix
transposes are ever needed; softmax denominators come from a ones-augmented
V stationary), and its partial output projection. Host sums the 4 partials
per batch and adds bo. No cross-core communication.

All matmuls run in bf16 (PE stays on the warm 2.4 GHz clock; fp32r's
transpose-mode path never warms the HAM clock gate and ran 2.7x slower).
"""

import numpy as np
import ml_dtypes

import concourse.bass as bass
import concourse.tile as tile
from concourse import bacc, mybir
from concourse.bass_utils import run_bass_kernel_spmd
from concourse.masks import make_identity

F32 = mybir.dt.float32
BF16 = mybir.dt.bfloat16

B, N, D = 2, 4096, 512
HEADS, DH = 8, 64
SCALE = DH ** -0.5          # 0.125
IC = 1024                   # i-chunk (query cols per psum-out accumulation)
N_IC = N // IC              # 4
N_JC = N // 128             # 32 j-chunks (key rows per matmul = partitions)
N_CORES = 8


def build_kernel():
    nc = bacc.Bacc("TRN2", target_bir_lowering=False, debug=False)
    xT_d = nc.dram_tensor("xT", [D, N], BF16, kind="ExternalInput").ap()
    wq_d = nc.dram_tensor("wq", [D, 128], BF16, kind="ExternalInput").ap()
    wk_d = nc.dram_tensor("wk", [D, 128], BF16, kind="ExternalInput").ap()
    wv_d = nc.dram_tensor("wv", [D, 128], BF16, kind="ExternalInput").ap()
    wo_d = nc.dram_tensor("wo", [128, D], BF16, kind="ExternalInput").ap()
    pT_d = nc.dram_tensor("pT", [D, N], F32, kind="ExternalOutput").ap()

    with tile.TileContext(nc) as tc:
        with (
            tc.tile_pool(name="const", bufs=1) as const_pool,
            tc.tile_pool(name="proj", bufs=1) as proj_pool,
            tc.tile_pool(name="pt", bufs=3) as pt_pool,
            tc.tile_pool(name="norm", bufs=2) as norm_pool,
            tc.tile_pool(name="stage", bufs=3) as stage_pool,
            tc.tile_pool(name="ps", bufs=2, space="PSUM") as ps_pool,
            tc.tile_pool(name="po", bufs=2, space="PSUM") as po_pool,
        ):
            # ---- P0: loads + constants -------------------------------------
            xt_sb = []
            for dc in range(4):
                t = const_pool.tile([128, N], BF16, name=f"xt{dc}", tag=f"xt{dc}")
                nc.sync.dma_start(t[:], xT_d[dc * 128:(dc + 1) * 128, :])
                xt_sb.append(t)
            w_sb = {}
            for nm, d_ap in (("wq", wq_d), ("wk", wk_d), ("wv", wv_d)):
                t = const_pool.tile([128, 4, 128], BF16, name=f"{nm}s", tag=f"{nm}s")
                nc.sync.dma_start(t[:], d_ap.rearrange("(c p) e -> p c e", p=128))
                w_sb[nm] = t
            wo_sb = const_pool.tile([128, D], BF16, name="wos", tag="wos")
            nc.sync.dma_start(wo_sb[:], wo_d[:])
            ident_f = const_pool.tile([128, 128], F32, name="ident_f",
                                      tag="ident_f")
            make_identity(nc, ident_f[:])
            ident = const_pool.tile([128, 128], BF16, name="ident", tag="ident")
            nc.vector.tensor_copy(ident[:], ident_f[:])

            # ---- P1: projections -------------------------------------------
            # Per-head q^T/k^T with K zero-padded to 128: head h occupies
            # partitions h*64..h*64+63, the other 64 partitions are zero.
            # K=64 matmuls never warm the PE HAM clock gate (measured 463 vs
            # 219 ns at K=128 for N=512), so we pay SBUF, not cycles.
            qTh = [proj_pool.tile([128, N], BF16, name=f"qTh{h}", tag=f"qTh{h}")
                   for h in range(2)]
            kTh = [proj_pool.tile([128, N], BF16, name=f"kTh{h}", tag=f"kTh{h}")
                   for h in range(2)]
            vT2 = proj_pool.tile([128, N], BF16, name="vT2", tag="vT2")
            for t in (qTh[0], kTh[0]):
                nc.vector.memset(t[64:128, :], 0.0)
            for t in (qTh[1], kTh[1]):
                nc.vector.memset(t[0:64, :], 0.0)
            for dsts, wname in ((kTh, "wk"), (None, "wv"), (qTh, "wq")):
                for i8 in range(8):
                    sl = slice(i8 * 512, (i8 + 1) * 512)
                    ps = ps_pool.tile([128, 1024], F32, name="ps", tag="ps")
                    for dc in range(4):
                        nc.tensor.matmul(
                            ps[:, 0:512],
                            w_sb[wname][:, dc, :],
                            xt_sb[dc][:, sl],
                            start=(dc == 0),
                            stop=(dc == 3),
                        )
                    if dsts is None:
                        nc.vector.tensor_copy(vT2[:, sl], ps[:, 0:512])
                    else:
                        nc.vector.tensor_copy(dsts[0][0:64, sl], ps[0:64, 0:512])
                        nc.vector.tensor_copy(dsts[1][64:128, sl],
                                              ps[64:128, 0:512])

            # v natural [j, e] in bf16, ones-augmented per head (ones column
            # FIRST so the softmax denominator lands on psum partition 0):
            # v2aug[:, jc, 0]=1, [1:65]=v_h0, [65]=1, [66:130]=v_h1
            v2aug = proj_pool.tile([128, N_JC, 130], BF16, name="v2aug",
                                   tag="v2aug")
            for jc in range(N_JC):
                psb = ps_pool.tile([128, 128], BF16, name="psb", tag="ps")
                nc.tensor.transpose(
                    psb[:, 0:128], vT2[:, jc * 128:(jc + 1) * 128], ident[:]
                )
                nc.vector.tensor_copy(v2aug[:, jc, 1:65], psb[:, 0:64])
                nc.vector.tensor_copy(v2aug[:, jc, 66:130], psb[:, 64:128])
            nc.vector.memset(v2aug[:, :, 0:1], 1.0)
            nc.vector.memset(v2aug[:, :, 65:66], 1.0)

            # ---- P2+P3: attention + normalize + output projection ----------
            for ic in range(N_IC):
                isl = slice(ic * IC, (ic + 1) * IC)
                outu = []
                for h in range(2):
                    pout = po_pool.tile([65, IC], F32, name="pout", tag="po")
                    for jc in range(N_JC):
                        jsl = slice(jc * 128, (jc + 1) * 128)
                        sc = ps_pool.tile([128, IC], F32, name="sc", tag="ps")
                        for n2 in range(2):
                            nsl = slice(n2 * 512, (n2 + 1) * 512)
                            nc.tensor.matmul(
                                sc[:, nsl],
                                kTh[h][:, jsl],
                                qTh[h][:, ic * IC + n2 * 512:
                                       ic * IC + (n2 + 1) * 512],
                                start=True,
                                stop=True,
                            )
                        pt = pt_pool.tile([128, IC], BF16, name="pt", tag="pt")
                        nc.scalar.activation(
                            pt[:], sc[:], mybir.ActivationFunctionType.Exp,
                            scale=SCALE,
                        )
                        for n2 in range(2):
                            nsl = slice(n2 * 512, (n2 + 1) * 512)
                            nc.tensor.matmul(
                                pout[:, nsl],
                                v2aug[:, jc, h * 65:h * 65 + 65],
                                pt[:, nsl],
                                start=(jc == 0),
                                stop=(jc == N_JC - 1),
                            )
                    # row 0 = denom, rows 1..64 = unnormalized out^T
                    ou = norm_pool.tile([65, IC], F32, name=f"outu{h}",
                                        tag=f"outu{h}")
                    nc.vector.tensor_copy(ou[:], pout[:])
                    outu.append(ou)

                # Normalize + stack both heads onto partitions 0..127:
                # denominators (row 0) partition-broadcast on GpSimd, out rows
                # (1..64) partition-shifted via SBUF->SBUF DMA, then
                # reciprocal + multiply on DVE.
                # partition_broadcast only writes correctly at out base 0, so
                # each head broadcasts into its own full-height tile and the
                # reciprocal reads the half that lines up with its out rows.
                den0 = norm_pool.tile([128, IC], F32, name="den0", tag="den0")
                den1 = norm_pool.tile([128, IC], F32, name="den1", tag="den1")
                st1 = norm_pool.tile([128, IC], F32, name="st1", tag="st1")
                for h, dtile in ((0, den0), (1, den1)):
                    psl = slice(h * 64, (h + 1) * 64)
                    nc.gpsimd.partition_broadcast(dtile[:, :], outu[h][0:1, :])
                    nc.sync.dma_start(st1[psl, :], outu[h][1:65, :])
                rec = norm_pool.tile([128, IC], F32, name="rec", tag="rec")
                nc.vector.reciprocal(rec[0:64, :], den0[0:64, :])
                nc.vector.reciprocal(rec[64:128, :], den1[64:128, :])
                outn = norm_pool.tile([128, IC], BF16, name="outn", tag="outn")
                nc.vector.tensor_mul(outn[:, :], st1[:, :], rec[:, :])

                # partial out projection: pT[oc, i] = wo[:, oc].T @ outn[:, i]
                for oc in range(4):
                    for n2 in range(2):
                        nsl = slice(n2 * 512, (n2 + 1) * 512)
                        pp = ps_pool.tile([128, 1024], F32, name="pp", tag="ps")
                        nc.tensor.matmul(
                            pp[:, 0:512],
                            wo_sb[:, oc * 128:(oc + 1) * 128],
                            outn[:, nsl],
                            start=True, stop=True,
                        )
                        st = stage_pool.tile([128, 512], F32, name="st",
                                             tag="st")
                        nc.vector.tensor_copy(st[:], pp[:, 0:512])
                        nc.sync.dma_start(
                            pT_d[oc * 128:(oc + 1) * 128,
                                 ic * IC + n2 * 512:ic * IC + (n2 + 1) * 512],
                            st[:],
                        )
    nc.compile()
    return nc


_CACHE = {}


def _get_nc():
    if "nc" not in _CACHE:
        _CACHE["nc"] = build_kernel()
    return _CACHE["nc"]


def make_in_map(x, Wq, Wkv, Wo, core):
    bf = ml_dtypes.bfloat16
    b, p = divmod(core, 4)
    cs = slice(128 * p, 128 * (p + 1))
    return {
        "xT": np.ascontiguousarray(x[b].T).astype(bf),
        "wq": np.ascontiguousarray(Wq[:, cs]).astype(bf),
        "wk": np.ascontiguousarray(Wkv[:, :D][:, cs]).astype(bf),
        "wv": np.ascontiguousarray(Wkv[:, D:][:, cs]).astype(bf),
        "wo": np.ascontiguousarray(Wo[cs, :]).astype(bf),
    }


def kernel(x, Wq, Wkv, Wo, bo):
    x = np.asarray(x, dtype=np.float32)
    Wq = np.asarray(Wq, dtype=np.float32)
    Wkv = np.asarray(Wkv, dtype=np.float32)
    Wo = np.asarray(Wo, dtype=np.float32)
    bo = np.asarray(bo, dtype=np.float32)

    nc = _get_nc()
    in_maps = [make_in_map(x, Wq, Wkv, Wo, c) for c in range(N_CORES)]
    res = run_bass_kernel_spmd(nc, in_maps, core_ids=list(range(N_CORES)))
    out = np.empty((B, N, D), dtype=np.float32)
    for b in range(B):
        acc = res.results[4 * b]["pT"].copy()
        for p in range(1, 4):
            acc += res.results[4 * b + p]["pT"]
        out[b] = acc.T + bo
    return out


# revision 22
# speedup vs baseline: 1.5811x; 1.0023x over previous
"""Multi-head self-attention (B=2, N=4096, D=512, h=8, d=64) on 8 TRN2 cores.

Sharding: batch*head-pair across the 8 cores (core c -> batch c//4, heads
2*(c%4), 2*(c%4)+1). Each core computes its two heads' q/k/v projections,
flash-style attention (scores kept transposed [j, i] so no P-matrix
transposes are ever needed; softmax denominators come from a ones-augmented
V stationary), and its partial output projection. Host sums the 4 partials
per batch and adds bo. No cross-core communication.

All matmuls run in bf16 (PE stays on the warm 2.4 GHz clock; fp32r's
transpose-mode path never warms the HAM clock gate and ran 2.7x slower).
"""

import numpy as np
import ml_dtypes

import concourse.bass as bass
import concourse.tile as tile
from concourse import bacc, mybir
from concourse.bass_utils import run_bass_kernel_spmd
from concourse.masks import make_identity

F32 = mybir.dt.float32
BF16 = mybir.dt.bfloat16

B, N, D = 2, 4096, 512
HEADS, DH = 8, 64
SCALE = DH ** -0.5          # 0.125
IC = 1024                   # i-chunk (query cols per psum-out accumulation)
N_IC = N // IC              # 4
N_JC = N // 128             # 32 j-chunks (key rows per matmul = partitions)
N_CORES = 8


def build_kernel():
    nc = bacc.Bacc("TRN2", target_bir_lowering=False, debug=False)
    xT_d = nc.dram_tensor("xT", [D, N], BF16, kind="ExternalInput").ap()
    wq_d = nc.dram_tensor("wq", [D, 128], BF16, kind="ExternalInput").ap()
    wk_d = nc.dram_tensor("wk", [D, 128], BF16, kind="ExternalInput").ap()
    wv_d = nc.dram_tensor("wv", [D, 128], BF16, kind="ExternalInput").ap()
    wo_d = nc.dram_tensor("wo", [128, D], BF16, kind="ExternalInput").ap()
    pT_d = nc.dram_tensor("pT", [D, N], F32, kind="ExternalOutput").ap()

    with tile.TileContext(nc) as tc:
        with (
            tc.tile_pool(name="const", bufs=1) as const_pool,
            tc.tile_pool(name="proj", bufs=1) as proj_pool,
            tc.tile_pool(name="pt", bufs=3) as pt_pool,
            tc.tile_pool(name="norm", bufs=2) as norm_pool,
            tc.tile_pool(name="stage", bufs=3) as stage_pool,
            tc.tile_pool(name="ps", bufs=2, space="PSUM") as ps_pool,
            tc.tile_pool(name="po", bufs=2, space="PSUM") as po_pool,
        ):
            # ---- P0: loads + constants -------------------------------------
            xt_sb = []
            for dc in range(4):
                t = const_pool.tile([128, N], BF16, name=f"xt{dc}", tag=f"xt{dc}")
                nc.sync.dma_start(t[:], xT_d[dc * 128:(dc + 1) * 128, :])
                xt_sb.append(t)
            w_sb = {}
            for nm, d_ap in (("wq", wq_d), ("wk", wk_d), ("wv", wv_d)):
                t = const_pool.tile([128, 4, 128], BF16, name=f"{nm}s", tag=f"{nm}s")
                nc.sync.dma_start(t[:], d_ap.rearrange("(c p) e -> p c e", p=128))
                w_sb[nm] = t
            wo_sb = const_pool.tile([128, D], BF16, name="wos", tag="wos")
            nc.sync.dma_start(wo_sb[:], wo_d[:])
            ident_f = const_pool.tile([128, 128], F32, name="ident_f",
                                      tag="ident_f")
            make_identity(nc, ident_f[:])
            ident = const_pool.tile([128, 128], BF16, name="ident", tag="ident")
            nc.vector.tensor_copy(ident[:], ident_f[:])

            # ---- P1: projections -------------------------------------------
            # Per-head q^T/k^T with K zero-padded to 128: head h occupies
            # partitions h*64..h*64+63, the other 64 partitions are zero.
            # K=64 matmuls never warm the PE HAM clock gate (measured 463 vs
            # 219 ns at K=128 for N=512), so we pay SBUF, not cycles.
            qTh = [proj_pool.tile([128, N], BF16, name=f"qTh{h}", tag=f"qTh{h}")
                   for h in range(2)]
            kTh = [proj_pool.tile([128, N], BF16, name=f"kTh{h}", tag=f"kTh{h}")
                   for h in range(2)]
            vT2 = proj_pool.tile([128, N], BF16, name="vT2", tag="vT2")
            for t in (qTh[0], kTh[0]):
                nc.vector.memset(t[64:128, :], 0.0)
            for t in (qTh[1], kTh[1]):
                nc.vector.memset(t[0:64, :], 0.0)
            # v natural [j, e] in bf16, ones-augmented per head (ones column
            # FIRST so the softmax denominator lands on psum partition 0):
            # v2aug[:, jc, 0]=1, [1:65]=v_h0, [65]=1, [66:130]=v_h1
            v2aug = proj_pool.tile([128, N_JC, 130], BF16, name="v2aug",
                                   tag="v2aug")
            nc.vector.memset(v2aug[:, :, 0:1], 1.0)
            nc.vector.memset(v2aug[:, :, 65:66], 1.0)

            def proj_chunk(wname, i8, dsts):
                sl = slice(i8 * 512, (i8 + 1) * 512)
                ps = ps_pool.tile([128, 1024], F32, name="ps", tag="ps")
                for dc in range(4):
                    nc.tensor.matmul(
                        ps[:, 0:512],
                        w_sb[wname][:, dc, :],
                        xt_sb[dc][:, sl],
                        start=(dc == 0),
                        stop=(dc == 3),
                    )
                if dsts is None:
                    nc.vector.tensor_copy(vT2[:, sl], ps[:, 0:512])
                else:
                    nc.vector.tensor_copy(dsts[0][0:64, sl], ps[0:64, 0:512])
                    nc.vector.tensor_copy(dsts[1][64:128, sl],
                                          ps[64:128, 0:512])

            # q's first i-chunks feed the very first scores matmuls, then k/v
            # interleave chunk-wise (with v transposes) so attention can start
            # while the projection tail is still running.
            for i8 in range(2):
                proj_chunk("wq", i8, qTh)
            for i8 in range(8):
                proj_chunk("wk", i8, kTh)
                proj_chunk("wv", i8, None)
                if i8 >= 2:
                    proj_chunk("wq", i8, qTh)
                for jc in range(4 * i8, 4 * i8 + 4):
                    psb = ps_pool.tile([128, 128], BF16, name="psb", tag="ps")
                    nc.tensor.transpose(
                        psb[:, 0:128], vT2[:, jc * 128:(jc + 1) * 128],
                        ident[:],
                    )
                    nc.vector.tensor_copy(v2aug[:, jc, 1:65], psb[:, 0:64])
                    nc.vector.tensor_copy(v2aug[:, jc, 66:130],
                                          psb[:, 64:128])

            # ---- P2+P3: attention + normalize + output projection ----------
            for ic in range(N_IC):
                isl = slice(ic * IC, (ic + 1) * IC)
                outu = []
                for h in range(2):
                    pout = po_pool.tile([65, IC], F32, name="pout", tag="po")
                    for jc in range(N_JC):
                        jsl = slice(jc * 128, (jc + 1) * 128)
                        sc = ps_pool.tile([128, IC], F32, name="sc", tag="ps")
                        for n2 in range(2):
                            nsl = slice(n2 * 512, (n2 + 1) * 512)
                            nc.tensor.matmul(
                                sc[:, nsl],
                                kTh[h][:, jsl],
                                qTh[h][:, ic * IC + n2 * 512:
                                       ic * IC + (n2 + 1) * 512],
                                start=True,
                                stop=True,
                            )
                        pt = pt_pool.tile([128, IC], BF16, name="pt", tag="pt")
                        nc.scalar.activation(
                            pt[:], sc[:], mybir.ActivationFunctionType.Exp,
                            scale=SCALE,
                        )
                        for n2 in range(2):
                            nsl = slice(n2 * 512, (n2 + 1) * 512)
                            nc.tensor.matmul(
                                pout[:, nsl],
                                v2aug[:, jc, h * 65:h * 65 + 65],
                                pt[:, nsl],
                                start=(jc == 0),
                                stop=(jc == N_JC - 1),
                            )
                    # row 0 = denom, rows 1..64 = unnormalized out^T
                    ou = norm_pool.tile([65, IC], F32, name=f"outu{h}",
                                        tag=f"outu{h}")
                    nc.vector.tensor_copy(ou[:], pout[:])
                    outu.append(ou)

                # Normalize + stack both heads onto partitions 0..127:
                # denominators (row 0) partition-broadcast on GpSimd, out rows
                # (1..64) partition-shifted via SBUF->SBUF DMA, then
                # reciprocal + multiply on DVE.
                # partition_broadcast only writes correctly at out base 0, so
                # each head broadcasts into its own full-height tile and the
                # reciprocal reads the half that lines up with its out rows.
                den0 = norm_pool.tile([128, IC], F32, name="den0", tag="den0")
                den1 = norm_pool.tile([128, IC], F32, name="den1", tag="den1")
                st1 = norm_pool.tile([128, IC], F32, name="st1", tag="st1")
                for h, dtile in ((0, den0), (1, den1)):
                    psl = slice(h * 64, (h + 1) * 64)
                    nc.gpsimd.partition_broadcast(dtile[:, :], outu[h][0:1, :])
                    nc.sync.dma_start(st1[psl, :], outu[h][1:65, :])
                rec = norm_pool.tile([128, IC], F32, name="rec", tag="rec")
                nc.vector.reciprocal(rec[0:64, :], den0[0:64, :])
                nc.vector.reciprocal(rec[64:128, :], den1[64:128, :])
                outn = norm_pool.tile([128, IC], BF16, name="outn", tag="outn")
                nc.vector.tensor_mul(outn[:, :], st1[:, :], rec[:, :])

                # partial out projection: pT[oc, i] = wo[:, oc].T @ outn[:, i]
                for oc in range(4):
                    for n2 in range(2):
                        nsl = slice(n2 * 512, (n2 + 1) * 512)
                        pp = po_pool.tile([128, 1024], F32, name="pp", tag="po")
                        nc.tensor.matmul(
                            pp[:, 0:512],
                            wo_sb[:, oc * 128:(oc + 1) * 128],
                            outn[:, nsl],
                            start=True, stop=True,
                        )
                        st = stage_pool.tile([128, 512], F32, name="st",
                                             tag="st")
                        nc.vector.tensor_copy(st[:], pp[:, 0:512])
                        nc.sync.dma_start(
                            pT_d[oc * 128:(oc + 1) * 128,
                                 ic * IC + n2 * 512:ic * IC + (n2 + 1) * 512],
                            st[:],
                        )
    nc.compile()
    return nc


_CACHE = {}


def _get_nc():
    if "nc" not in _CACHE:
        _CACHE["nc"] = build_kernel()
    return _CACHE["nc"]


def make_in_map(x, Wq, Wkv, Wo, core):
    bf = ml_dtypes.bfloat16
    b, p = divmod(core, 4)
    cs = slice(128 * p, 128 * (p + 1))
    return {
        "xT": np.ascontiguousarray(x[b].T).astype(bf),
        "wq": np.ascontiguousarray(Wq[:, cs]).astype(bf),
        "wk": np.ascontiguousarray(Wkv[:, :D][:, cs]).astype(bf),
        "wv": np.ascontiguousarray(Wkv[:, D:][:, cs]).astype(bf),
        "wo": np.ascontiguousarray(Wo[cs, :]).astype(bf),
    }


def kernel(x, Wq, Wkv, Wo, bo):
    x = np.asarray(x, dtype=np.float32)
    Wq = np.asarray(Wq, dtype=np.float32)
    Wkv = np.asarray(Wkv, dtype=np.float32)
    Wo = np.asarray(Wo, dtype=np.float32)
    bo = np.asarray(bo, dtype=np.float32)

    nc = _get_nc()
    in_maps = [make_in_map(x, Wq, Wkv, Wo, c) for c in range(N_CORES)]
    res = run_bass_kernel_spmd(nc, in_maps, core_ids=list(range(N_CORES)))
    out = np.empty((B, N, D), dtype=np.float32)
    for b in range(B):
        acc = res.results[4 * b]["pT"].copy()
        for p in range(1, 4):
            acc += res.results[4 * b + p]["pT"]
        out[b] = acc.T + bo
    return out
